# revision 8
# baseline (speedup 1.0000x reference)
"""Trainium2 Bass kernel for nn_Attention_aggregator (B=8, N=4096, F=128, E=128).

Sharding: data-parallel over batch — one batch element per NeuronCore (8 cores).
Each core computes, for its batch b:
    att  = x @ x.T                        [N, N]   (symmetric)
    att  = where(adj==0, -9999999, att)
    sm   = softmax(att, axis=-1)
    comb = sm @ x                         [N, F]
    out  = relu(concat([x, comb], -1) @ W.T)      [N, E]

Device decomposition (works in the transposed orientation so the aggregation
matmul's contraction dim lands on partitions; attention symmetry makes the
transposed logits free):
    E^T[m, r] = exp(att[m, r] - 80)       (att[m,r] == att[r,m])
    diagonal of att killed in PSUM by an accumulating (-30000*I) @ I matmul
    P^T = E^T * adjT                      (adjT loaded as int32; mixed-dtype mult)
    [S2 | S1] = P^T.T @ [x | 1]           (ones column => row-sum in column F)
    comb = (ev*S2 + coef*x) / (ev*S1 + coef)   with d = ||x_r||^2,
        ev = exp(-adj_rr*max(0, d-110)),  coef = adj_rr*exp(min(d-80, 30))
    (diagonal handled analytically: its logit is the only one that can
     overflow exp; everything off-diagonal is bounded ~|att|<70)
    out = relu([x, comb] @ W.T) with comb'^T stationary matmuls.

Host-side prep inside kernel() is layout-only: adj is transposed per batch
(values unchanged, int32); the device still streams the full 64MB adjacency
per core. The adjacency diagonal (4096 int32 per batch) is also passed
separately in a DMA-friendly layout.
"""

import sys

for _p in ("/opt/trn_rl_repo", "/root/.axon_site/_ro/trn_rl_repo"):
    if _p not in sys.path:
        sys.path.append(_p)

import numpy as np

import concourse.bass as bass
import concourse.mybir as mybir
from concourse import bacc
from concourse.tile import TileContext
from concourse.masks import make_identity
from concourse.bass_utils import run_bass_kernel_spmd

F32 = mybir.dt.float32
BF16 = mybir.dt.bfloat16
I32 = mybir.dt.int32
I16 = mybir.dt.int16

B, N, F, E = 8, 4096, 128, 128
RC = 512               # r-chunk width (one PSUM bank of fp32)
NB = N // 128          # 32 m-blocks
NRC = N // RC          # 8 r-chunks
T = RC // 128          # 4 sub-blocks per r-chunk
EXP_BIAS = -80.0

_CACHED = {}


def _build():
    nc = bacc.Bacc("TRN2", target_bir_lowering=False, debug=False, num_devices=B)
    x_d = nc.dram_tensor("x", [N, F], F32, kind="ExternalInput").ap()
    adjt_d = nc.dram_tensor("adjt", [N, N], I32, kind="ExternalInput").ap()
    adjd_d = nc.dram_tensor("adjd", [128, NB], F32, kind="ExternalInput").ap()
    w_d = nc.dram_tensor("w", [E, 2 * F], F32, kind="ExternalInput").ap()
    out_d = nc.dram_tensor("out", [N, E], F32, kind="ExternalOutput").ap()

    x_v = x_d.rearrange("(o p) f -> p o f", p=128)          # [128, NB, F]
    adjt_v = adjt_d.rearrange("(o p) c -> p o c", p=128)    # [128, NB, N]
    w_v = w_d.rearrange("e (h f) -> e h f", h=2)            # [128, 2, F]
    out_v = out_d.rearrange("(o p) e -> p o e", p=128)      # [128, NB, E]

    with TileContext(nc) as tc:
        with (
            tc.tile_pool(name="singles", bufs=1) as singles,
            tc.tile_pool(name="adj", bufs=4) as adj_pool,
            tc.tile_pool(name="et", bufs=5) as e_pool,
            tc.tile_pool(name="pt", bufs=5) as p_pool,
            tc.tile_pool(name="small", bufs=12) as small,
            tc.tile_pool(name="outp", bufs=6) as out_pool,
            tc.tile_pool(name="psumA", bufs=2, space="PSUM") as psum_a,
            tc.tile_pool(name="psumC", bufs=1, space="PSUM") as psum_c,
            tc.tile_pool(name="psumT", bufs=2, space="PSUM") as psum_t,
        ):
            # ---------------- setup ----------------
            x_sb = singles.tile([128, NB, F], F32)
            nc.sync.dma_start(out=x_sb[:], in_=x_v)

            expb = singles.tile([128, 1], F32)
            nc.vector.memset(expb[:], EXP_BIAS)

            ident = singles.tile([128, 128], F32)
            make_identity(nc, ident)
            ident_bf = singles.tile([128, 128], BF16)
            nc.vector.tensor_copy(ident_bf[:], ident[:])
            negbig_bf = singles.tile([128, 128], BF16)
            nc.vector.tensor_scalar_mul(negbig_bf[:], ident_bf[:], -30000.0)

            # bf16 x with ones column (moving operand of the aggregation matmul)
            xb_sb = singles.tile([128, NB, F + 4], BF16)
            nc.vector.tensor_copy(xb_sb[:, :, :F], x_sb[:])
            nc.vector.memset(xb_sb[:, :, F:F + 1], 1.0)

            # x^T bf16 [f part, m free] via PE transposes of the bf16 x
            xt_sb = singles.tile([128, NB, 128], BF16)
            for j in range(NB):
                psb = psum_t.tile([128, 128], BF16, tag="tr")
                nc.tensor.transpose(psb[:], xb_sb[:, j, 0:128], ident_bf[:])
                nc.vector.tensor_copy(xt_sb[:, j, :], psb[:])

            # W^T fp32 halves [f part, e free]
            w_sb = singles.tile([128, 2, F], F32)
            nc.sync.dma_start(out=w_sb[:], in_=w_v)
            wb_sb = singles.tile([128, 2, F], BF16)
            nc.vector.tensor_copy(wb_sb[:], w_sb[:])
            wt_sb = singles.tile([128, 2, E], BF16)
            for h in range(2):
                psb = psum_t.tile([128, 128], BF16, tag="tr")
                nc.tensor.transpose(psb[:], wb_sb[:, h, :], ident_bf[:])
                nc.vector.tensor_copy(wt_sb[:, h, :], psb[:])

            # adjacency diagonal flags [128, NB] fp32 (host-prepacked layout)
            adjd_sb = singles.tile([128, NB], F32)
            nc.sync.dma_start(out=adjd_sb[:], in_=adjd_d)

            # d_r = ||x_r||^2 per row -> [128, NB]
            d_sb = singles.tile([128, NB], F32)
            for j in range(NB):
                scr = small.tile([128, F], F32, tag="sq_scr")
                nc.scalar.activation(scr[:], x_sb[:, j, :],
                                     mybir.ActivationFunctionType.Square,
                                     accum_out=d_sb[:, j:j + 1])

            # ev = exp(-adj_rr*max(0, d-110)); coef = adj_rr*exp(min(d-80, 30))
            ev_sb = singles.tile([128, NB], F32)
            coef_sb = singles.tile([128, NB], F32)
            t1 = small.tile([128, NB], F32, tag="cor")
            nc.vector.tensor_scalar(t1[:], d_sb[:], -110.0, 0.0,
                                    mybir.AluOpType.add, mybir.AluOpType.max)
            t2 = small.tile([128, NB], F32, tag="cor")
            nc.vector.tensor_tensor(t2[:], t1[:], adjd_sb[:], mybir.AluOpType.mult)
            nc.scalar.activation(ev_sb[:], t2[:],
                                 mybir.ActivationFunctionType.Exp, scale=-1.0)
            t3 = small.tile([128, NB], F32, tag="cor")
            nc.vector.tensor_scalar(t3[:], d_sb[:], -80.0, 30.0,
                                    mybir.AluOpType.add, mybir.AluOpType.min)
            t4 = small.tile([128, NB], F32, tag="cor")
            nc.scalar.activation(t4[:], t3[:], mybir.ActivationFunctionType.Exp)
            nc.vector.tensor_tensor(coef_sb[:], t4[:], adjd_sb[:],
                                    mybir.AluOpType.mult)

            # ---------------- main loop ----------------
            for rc in range(NRC):
                psumC = [psum_c.tile([128, F + 1], F32, name=f"psumC{t}",
                                     tag=f"psumC{t}") for t in range(T)]
                ajt_big = None
                for j in range(NB):
                    if j % 4 == 0:
                        ajt_big = adj_pool.tile([128, 4, RC], I16, name="ajt_big")
                        nc.gpsimd.dma_start(
                            out=ajt_big[:],
                            in_=adjt_v[:, j:j + 4, rc * RC:(rc + 1) * RC])

                    psA = psum_a.tile([128, RC], F32)
                    diag = rc * T <= j < (rc + 1) * T
                    nc.tensor.matmul(psA[:], xt_sb[:, j, :],
                                     xt_sb[:, rc * T:(rc + 1) * T, :],
                                     start=True, stop=not diag,
                                     skip_group_check=True)
                    if diag:
                        off = (j - rc * T) * 128
                        nc.tensor.matmul(psA[:, off:off + 128],
                                         negbig_bf[:], ident_bf[:],
                                         start=False, stop=True,
                                         skip_group_check=True)

                    et = e_pool.tile([128, RC], BF16)
                    nc.scalar.activation(et[:], psA[:],
                                         mybir.ActivationFunctionType.Exp,
                                         bias=expb[:])
                    pt = p_pool.tile([128, RC], BF16)
                    nc.vector.tensor_tensor(
                        pt[:], et[:], ajt_big[:, j % 4, :],
                        mybir.AluOpType.mult)
                    for t in range(T):
                        nc.tensor.matmul(psumC[t][:],
                                         pt[:, t * 128:(t + 1) * 128],
                                         xb_sb[:, j, 0:F + 1],
                                         start=(j == 0), stop=(j == NB - 1))

                sc = [small.tile([128, F + 1], F32, name=f"sc{t}",
                                 tag=f"sc{t}") for t in range(T)]
                for t in range(T):
                    nc.vector.tensor_copy(sc[t][:], psumC[t][:])
                for t in range(T):
                    blk = rc * T + t
                    evb = ev_sb[:, blk:blk + 1]
                    cfb = coef_sb[:, blk:blk + 1]
                    den = small.tile([128, 1], F32, tag="den")
                    nc.vector.scalar_tensor_tensor(
                        den[:], sc[t][:, F:F + 1], evb, cfb,
                        mybir.AluOpType.mult, mybir.AluOpType.add)
                    rden = small.tile([128, 1], F32, tag="rden")
                    nc.vector.reciprocal(rden[:], den[:])
                    xs = small.tile([128, F], F32, tag="xs")
                    nc.vector.tensor_scalar_mul(xs[:], x_sb[:, blk, :], cfb)
                    cu = small.tile([128, F], F32, tag="cu")
                    nc.vector.scalar_tensor_tensor(
                        cu[:], sc[t][:, 0:F], evb, xs[:],
                        mybir.AluOpType.mult, mybir.AluOpType.add)
                    cn = small.tile([128, F], BF16, tag="cn")
                    nc.vector.tensor_scalar_mul(cn[:], cu[:], rden[:])

                    psT = psum_t.tile([128, 128], BF16, tag="tr")
                    nc.tensor.transpose(psT[:], cn[:], ident_bf[:])
                    cnT = small.tile([128, F], BF16, tag="cnT")
                    nc.vector.tensor_copy(cnT[:], psT[:])

                    psF = psum_t.tile([128, E], F32, tag="tr")
                    nc.tensor.matmul(psF[:], xt_sb[:, blk, :], wt_sb[:, 0, :],
                                     start=True, stop=False)
                    nc.tensor.matmul(psF[:], cnT[:], wt_sb[:, 1, :],
                                     start=False, stop=True)
                    ot = out_pool.tile([128, E], F32)
                    nc.vector.tensor_relu(ot[:], psF[:])
                    nc.sync.dma_start(out=out_v[:, blk, :], in_=ot[:])

    nc.compile()
    return nc


def _get_nc():
    if "nc" not in _CACHED:
        _CACHED["nc"] = _build()
    return _CACHED["nc"]


def kernel(**inputs) -> np.ndarray:
    x_all = np.asarray(inputs["node_features"], dtype=np.float32)   # [B, N, F]
    adj_all = np.asarray(inputs["adj_list"])                        # [B, N, N] int32
    W = np.asarray(inputs["W"], dtype=np.float32)                   # [E, 2F]

    nc = _get_nc()
    in_maps = []
    for b in range(B):
        adjt = np.ascontiguousarray(adj_all[b].T).astype(np.int32, copy=False)
        diag = np.ascontiguousarray(np.diagonal(adj_all[b])).astype(np.float32)
        adjd = np.ascontiguousarray(diag.reshape(NB, 128).T)
        in_maps.append({
            "x": np.ascontiguousarray(x_all[b]),
            "adjt": adjt,
            "adjd": adjd,
            "w": W,
        })

    res = run_bass_kernel_spmd(nc, in_maps, core_ids=list(range(B)))
    out = np.stack([res.results[b]["out"] for b in range(B)], axis=0)
    return out.astype(np.float32, copy=False)


# revision 9
# speedup vs baseline: 1.1089x; 1.1089x over previous
"""Trainium2 Bass kernel for nn_Attention_aggregator (B=8, N=4096, F=128, E=128).

Sharding: data-parallel over batch — one batch element per NeuronCore (8 cores).
Each core computes, for its batch b:
    att  = x @ x.T                        [N, N]   (symmetric)
    att  = where(adj==0, -9999999, att)
    sm   = softmax(att, axis=-1)
    comb = sm @ x                         [N, F]
    out  = relu(concat([x, comb], -1) @ W.T)      [N, E]

Device decomposition (works in the transposed orientation so the aggregation
matmul's contraction dim lands on partitions; attention symmetry makes the
transposed logits free):
    E^T[m, r] = exp(att[m, r] - 80)       (att[m,r] == att[r,m])
    diagonal of att killed in PSUM by an accumulating (-30000*I) @ I matmul
    P^T = E^T * adjT                      (adjT loaded as int32; mixed-dtype mult)
    [S2 | S1] = P^T.T @ [x | 1]           (ones column => row-sum in column F)
    comb = (ev*S2 + coef*x) / (ev*S1 + coef)   with d = ||x_r||^2,
        ev = exp(-adj_rr*max(0, d-110)),  coef = adj_rr*exp(min(d-80, 30))
    (diagonal handled analytically: its logit is the only one that can
     overflow exp; everything off-diagonal is bounded ~|att|<70)
    out = relu([x, comb] @ W.T) with comb'^T stationary matmuls.

Host-side prep inside kernel() is layout-only: adj is transposed per batch
(values unchanged, int32); the device still streams the full 64MB adjacency
per core. The adjacency diagonal (4096 int32 per batch) is also passed
separately in a DMA-friendly layout.
"""

import sys

for _p in ("/opt/trn_rl_repo", "/root/.axon_site/_ro/trn_rl_repo"):
    if _p not in sys.path:
        sys.path.append(_p)

import numpy as np

import concourse.bass as bass
import concourse.mybir as mybir
from concourse import bacc
from concourse.tile import TileContext
from concourse.masks import make_identity
from concourse.bass_utils import run_bass_kernel_spmd

F32 = mybir.dt.float32
BF16 = mybir.dt.bfloat16
I32 = mybir.dt.int32
I16 = mybir.dt.int16

B, N, F, E = 8, 4096, 128, 128
RC = 512               # r-chunk width (one PSUM bank of fp32)
NB = N // 128          # 32 m-blocks
NRC = N // RC          # 8 r-chunks
T = RC // 128          # 4 sub-blocks per r-chunk
EXP_BIAS = -80.0

_CACHED = {}


def _build():
    nc = bacc.Bacc("TRN2", target_bir_lowering=False, debug=False, num_devices=B)
    x_d = nc.dram_tensor("x", [128, NB, F], F32, kind="ExternalInput").ap()
    adjt_d = nc.dram_tensor("adjt", [N, N], I32, kind="ExternalInput").ap()
    adjd_d = nc.dram_tensor("adjd", [128, NB], F32, kind="ExternalInput").ap()
    w_d = nc.dram_tensor("w", [E, 2 * F], F32, kind="ExternalInput").ap()
    out_d = nc.dram_tensor("out", [N, E], F32, kind="ExternalOutput").ap()

    x_v = x_d  # host-shuffled to [128, NB, F] (m = o*128 + p)
    adjt_v = adjt_d.rearrange("(o p) c -> p o c", p=128)    # [128, NB, N]
    w_v = w_d.rearrange("e (h f) -> e h f", h=2)            # [128, 2, F]
    out_v = out_d.rearrange("(o p) e -> p o e", p=128)      # [128, NB, E]

    with TileContext(nc) as tc:
        with (
            tc.tile_pool(name="singles", bufs=1) as singles,
            tc.tile_pool(name="adj", bufs=4) as adj_pool,
            tc.tile_pool(name="et", bufs=5) as e_pool,
            tc.tile_pool(name="pt", bufs=5) as p_pool,
            tc.tile_pool(name="small", bufs=12) as small,
            tc.tile_pool(name="outp", bufs=6) as out_pool,
            tc.tile_pool(name="psumA", bufs=2, space="PSUM") as psum_a,
            tc.tile_pool(name="psumC", bufs=1, space="PSUM") as psum_c,
            tc.tile_pool(name="psumT", bufs=2, space="PSUM") as psum_t,
        ):
            # ---------------- setup ----------------
            x_sb = singles.tile([128, NB, F], F32)
            nc.sync.dma_start(out=x_sb[:], in_=x_v)

            expb = singles.tile([128, 1], F32)
            nc.vector.memset(expb[:], EXP_BIAS)

            ident = singles.tile([128, 128], F32)
            make_identity(nc, ident)
            ident_bf = singles.tile([128, 128], BF16)
            nc.vector.tensor_copy(ident_bf[:], ident[:])
            negbig_bf = singles.tile([128, 128], BF16)
            nc.vector.tensor_scalar_mul(negbig_bf[:], ident_bf[:], -30000.0)

            # bf16 x with ones column (moving operand of the aggregation matmul)
            xb_sb = singles.tile([128, NB, F + 4], BF16)
            nc.vector.tensor_copy(xb_sb[:, :, :F], x_sb[:])
            nc.vector.memset(xb_sb[:, :, F:F + 1], 1.0)

            # x^T bf16 [f part, m free] via PE transposes of the bf16 x
            xt_sb = singles.tile([128, NB, 128], BF16)
            for j in range(NB):
                psb = psum_t.tile([128, 128], BF16, tag="tr")
                nc.tensor.transpose(psb[:], xb_sb[:, j, 0:128], ident_bf[:])
                nc.vector.tensor_copy(xt_sb[:, j, :], psb[:])

            # W^T fp32 halves [f part, e free]
            w_sb = singles.tile([128, 2, F], F32)
            nc.sync.dma_start(out=w_sb[:], in_=w_v)
            wb_sb = singles.tile([128, 2, F], BF16)
            nc.vector.tensor_copy(wb_sb[:], w_sb[:])
            wt_sb = singles.tile([128, 2, E], BF16)
            for h in range(2):
                psb = psum_t.tile([128, 128], BF16, tag="tr")
                nc.tensor.transpose(psb[:], wb_sb[:, h, :], ident_bf[:])
                nc.vector.tensor_copy(wt_sb[:, h, :], psb[:])

            # adjacency diagonal flags [128, NB] fp32 (host-prepacked layout)
            adjd_sb = singles.tile([128, NB], F32)
            nc.sync.dma_start(out=adjd_sb[:], in_=adjd_d)

            # d_r = ||x_r||^2 per row -> [128, NB]
            d_sb = singles.tile([128, NB], F32)
            for j in range(NB):
                scr = small.tile([128, F], F32, tag="sq_scr")
                nc.scalar.activation(scr[:], x_sb[:, j, :],
                                     mybir.ActivationFunctionType.Square,
                                     accum_out=d_sb[:, j:j + 1])

            # ev = exp(-adj_rr*max(0, d-110)); coef = adj_rr*exp(min(d-80, 30))
            ev_sb = singles.tile([128, NB], F32)
            coef_sb = singles.tile([128, NB], F32)
            t1 = small.tile([128, NB], F32, tag="cor")
            nc.vector.tensor_scalar(t1[:], d_sb[:], -110.0, 0.0,
                                    mybir.AluOpType.add, mybir.AluOpType.max)
            t2 = small.tile([128, NB], F32, tag="cor")
            nc.vector.tensor_tensor(t2[:], t1[:], adjd_sb[:], mybir.AluOpType.mult)
            nc.scalar.activation(ev_sb[:], t2[:],
                                 mybir.ActivationFunctionType.Exp, scale=-1.0)
            t3 = small.tile([128, NB], F32, tag="cor")
            nc.vector.tensor_scalar(t3[:], d_sb[:], -80.0, 30.0,
                                    mybir.AluOpType.add, mybir.AluOpType.min)
            t4 = small.tile([128, NB], F32, tag="cor")
            nc.scalar.activation(t4[:], t3[:], mybir.ActivationFunctionType.Exp)
            nc.vector.tensor_tensor(coef_sb[:], t4[:], adjd_sb[:],
                                    mybir.AluOpType.mult)

            # ---------------- main loop ----------------
            for rc in range(NRC):
                psumC = [psum_c.tile([128, F + 1], F32, name=f"psumC{t}",
                                     tag=f"psumC{t}") for t in range(T)]
                ajt_big = None
                for j in range(NB):
                    if j % 4 == 0:
                        ajt_big = adj_pool.tile([128, 4, RC], I16, name="ajt_big")
                        nc.gpsimd.dma_start(
                            out=ajt_big[:],
                            in_=adjt_v[:, j:j + 4, rc * RC:(rc + 1) * RC])

                    psA = psum_a.tile([128, RC], F32)
                    diag = rc * T <= j < (rc + 1) * T
                    nc.tensor.matmul(psA[:], xt_sb[:, j, :],
                                     xt_sb[:, rc * T:(rc + 1) * T, :],
                                     start=True, stop=not diag,
                                     skip_group_check=True)
                    if diag:
                        off = (j - rc * T) * 128
                        nc.tensor.matmul(psA[:, off:off + 128],
                                         negbig_bf[:], ident_bf[:],
                                         start=False, stop=True,
                                         skip_group_check=True)

                    et = e_pool.tile([128, RC], BF16)
                    nc.scalar.activation(et[:], psA[:],
                                         mybir.ActivationFunctionType.Exp,
                                         bias=expb[:])
                    pt = p_pool.tile([128, RC], BF16)
                    nc.vector.tensor_tensor(
                        pt[:], et[:], ajt_big[:, j % 4, :],
                        mybir.AluOpType.mult)
                    for t in range(T):
                        nc.tensor.matmul(psumC[t][:],
                                         pt[:, t * 128:(t + 1) * 128],
                                         xb_sb[:, j, 0:F + 1],
                                         start=(j == 0), stop=(j == NB - 1))

                sc = [small.tile([128, F + 1], F32, name=f"sc{t}",
                                 tag=f"sc{t}") for t in range(T)]
                for t in range(T):
                    nc.vector.tensor_copy(sc[t][:], psumC[t][:])
                for t in range(T):
                    blk = rc * T + t
                    evb = ev_sb[:, blk:blk + 1]
                    cfb = coef_sb[:, blk:blk + 1]
                    den = small.tile([128, 1], F32, tag="den")
                    nc.vector.scalar_tensor_tensor(
                        den[:], sc[t][:, F:F + 1], evb, cfb,
                        mybir.AluOpType.mult, mybir.AluOpType.add)
                    rden = small.tile([128, 1], F32, tag="rden")
                    nc.vector.reciprocal(rden[:], den[:])
                    xs = small.tile([128, F], F32, tag="xs")
                    nc.vector.tensor_scalar_mul(xs[:], x_sb[:, blk, :], cfb)
                    cu = small.tile([128, F], F32, tag="cu")
                    nc.vector.scalar_tensor_tensor(
                        cu[:], sc[t][:, 0:F], evb, xs[:],
                        mybir.AluOpType.mult, mybir.AluOpType.add)
                    cn = small.tile([128, F], BF16, tag="cn")
                    nc.vector.tensor_scalar_mul(cn[:], cu[:], rden[:])

                    psT = psum_t.tile([128, 128], BF16, tag="tr")
                    nc.tensor.transpose(psT[:], cn[:], ident_bf[:])
                    cnT = small.tile([128, F], BF16, tag="cnT")
                    nc.vector.tensor_copy(cnT[:], psT[:])

                    psF = psum_t.tile([128, E], F32, tag="tr")
                    nc.tensor.matmul(psF[:], xt_sb[:, blk, :], wt_sb[:, 0, :],
                                     start=True, stop=False)
                    nc.tensor.matmul(psF[:], cnT[:], wt_sb[:, 1, :],
                                     start=False, stop=True)
                    ot = out_pool.tile([128, E], F32)
                    nc.vector.tensor_relu(ot[:], psF[:])
                    nc.sync.dma_start(out=out_v[:, blk, :], in_=ot[:])

    nc.compile()
    return nc


def _get_nc():
    if "nc" not in _CACHED:
        _CACHED["nc"] = _build()
    return _CACHED["nc"]


def kernel(**inputs) -> np.ndarray:
    x_all = np.asarray(inputs["node_features"], dtype=np.float32)   # [B, N, F]
    adj_all = np.asarray(inputs["adj_list"])                        # [B, N, N] int32
    W = np.asarray(inputs["W"], dtype=np.float32)                   # [E, 2F]

    nc = _get_nc()
    in_maps = []
    for b in range(B):
        adjt = np.ascontiguousarray(adj_all[b].T).astype(np.int32, copy=False)
        diag = np.ascontiguousarray(np.diagonal(adj_all[b])).astype(np.float32)
        adjd = np.ascontiguousarray(diag.reshape(NB, 128).T)
        xshuf = np.ascontiguousarray(
            x_all[b].reshape(NB, 128, F).transpose(1, 0, 2))
        in_maps.append({
            "x": xshuf,
            "adjt": adjt,
            "adjd": adjd,
            "w": W,
        })

    res = run_bass_kernel_spmd(nc, in_maps, core_ids=list(range(B)))
    out = np.stack([res.results[b]["out"] for b in range(B)], axis=0)
    return out.astype(np.float32, copy=False)
